# revision 9
# baseline (speedup 1.0000x reference)
"""FBSNN net_u_Du kernel for 8 trn2 NeuronCores.

Computes, for u(s) = W2 @ sin(W1 @ s + b1) + b2 with s = [t, x]:
  u            (M,1)
  DuDx = g[:,1:], DuDt = g[:,:1]  with  g = (W2 o cos Z) @ W1
  D2uDx2[m]    = V^T diag(-W2 o sin z_m) V,  V = W1[:,1:]

Key reduction: the per-sample Hessians batch into one dense matmul
  D2[m, j*100+k] = sum_h S[h,m] * P[h, j*100+k]
with S[h,m] = W2[h]*sin(Z[m,h]) and P[h,jk] = -V[h,j]*V[h,k].
Data parallel over M=4096 paths -> 512 per core; weights replicated.

HW notes this shape leans on:
 - HW Sin is only accurate on [-pi, pi]; arguments are range-reduced with
   w = y - 2pi*round(y/2pi) (the DVE f32->i32 cast rounds to nearest).
 - fp32 matmul runs as two PE passes; the Hessian matmul dtype is
   switchable below (fp16 operands stream at full rate, fp32 is exact).
 - inputs are packed into 2 DRAM tensors so startup is 3 large DMAs, not
   12 tiny serialized ones.
"""

import numpy as np

import concourse.bacc as bacc
import concourse.mybir as mybir
import concourse.tile as tile
from concourse.bass_utils import run_bass_kernel_spmd

N_CORES = 8
M_FULL = 4096
MC = M_FULL // N_CORES  # 512 paths per core
D = 100
DP1 = D + 1  # 101
H = 256  # hidden width
F32 = mybir.dt.float32
F16 = mybir.dt.float16

HESS_FP32 = True  # False: fp16 Hessian matmul (2x faster PE, ~5e-4 D2 err)

NCHUNK = 500  # matmul free-dim per Hessian matmul (<=512 fp32 PSUM bank)
CPG = 4  # chunks per DMA group
GCOLS = NCHUNK * CPG  # 2000
NGROUPS = (D * D) // GCOLS  # 5
NM = MC // 128  # 4 m-chunks of 128 paths

# set by test harness to profile; kernel() records exec time here
TRACE = False
LAST_EXEC_NS = None

_CACHE = {}


def _build():
    nc = bacc.Bacc(None, target_bir_lowering=False, debug=False)
    sin_f = mybir.ActivationFunctionType.Sin
    mult = mybir.AluOpType.mult
    hdt = F32 if HESS_FP32 else F16

    # CA = [XT | W1T] on 101 partitions; CB = [W1 | b1 | W2 | b2pad] on 256.
    ca_d = nc.dram_tensor("CA", [DP1, MC + H], F32, kind="ExternalInput")
    cb_d = nc.dram_tensor("CB", [H, DP1 + 3], F32, kind="ExternalInput")

    u_d = nc.dram_tensor("u", [1, MC], F32, kind="ExternalOutput")
    dudt_d = nc.dram_tensor("DuDt", [MC, 1], F32, kind="ExternalOutput")
    dudx_d = nc.dram_tensor("DuDx", [MC, D], F32, kind="ExternalOutput")
    d2_d = nc.dram_tensor("D2", [MC, D * D], F32, kind="ExternalOutput")

    with tile.TileContext(nc) as tc:
        with (
            tc.tile_pool(name="const", bufs=1) as const,
            tc.tile_pool(name="work", bufs=2) as work,
            tc.tile_pool(name="pP", bufs=3) as pP,
            tc.tile_pool(name="stage", bufs=4) as stage_p,
        ):
            # ---- load inputs: 3 large DMAs ----
            ca = const.tile([DP1, MC + H], F32)
            nc.sync.dma_start(ca[:], ca_d[:])
            cb = [const.tile([128, DP1 + 3], F32, tag=f"cb_{k}", name=f"cb_{k}") for k in range(2)]
            for k in range(2):
                nc.sync.dma_start(cb[k][:], cb_d[k * 128 : (k + 1) * 128, :])
            xt = ca[:, 0:MC]
            w1t = ca[:, MC : MC + H]
            w1 = [cb[k][:, 0:DP1] for k in range(2)]
            b1c = [cb[k][:, DP1 : DP1 + 1] for k in range(2)]
            w2c = [cb[k][:, DP1 + 1 : DP1 + 2] for k in range(2)]
            b2t = cb[0][0:1, DP1 + 2 : DP1 + 3]

            ones = const.tile([128, 1], F32)
            nc.vector.memset(ones[:], 1.0)
            pihalf = const.tile([128, 1], F32)
            nc.vector.memset(pihalf[:], float(np.pi / 2))

            # hess-dtype views of V = W1[:, 1:] and -V
            vv = [const.tile([128, D], hdt, tag=f"vv_{k}", name=f"vv_{k}") for k in range(2)]
            negv = [const.tile([128, D], hdt, tag=f"negv_{k}", name=f"negv_{k}") for k in range(2)]
            for k in range(2):
                nc.vector.tensor_copy(vv[k][:], w1[k][:, 1:DP1])
                nc.vector.tensor_scalar_mul(negv[k][:], w1[k][:, 1:DP1], -1.0)

            S32 = [const.tile([128, MC], F32, tag=f"S32_{k}", name=f"S32_{k}") for k in range(2)]
            SH = S32 if HESS_FP32 else [
                const.tile([128, MC], F16, tag=f"S16_{k}", name=f"S16_{k}") for k in range(2)
            ]
            C = [const.tile([128, MC], F32, tag=f"C_{k}", name=f"C_{k}") for k in range(2)]
            inv2pi = float(1.0 / (2.0 * np.pi))
            twopi = float(2.0 * np.pi)

            with tc.tile_pool(name="psE", bufs=2, space="PSUM") as psE:
                # ---- trig: Z^T = W1 @ [t,X]^T; S = W2*sin(Z), C = W2*cos(Z) ----
                for k in range(2):
                    ztp = psE.tile([128, MC], F32, tag="zt")
                    nc.tensor.matmul(
                        ztp[:], w1t[:, k * 128 : (k + 1) * 128], xt[:],
                        start=True, stop=True,
                    )
                    y = work.tile([128, MC], F32, tag="y")
                    nc.vector.tensor_scalar_add(y[:], ztp[:], b1c[k])
                    ki = work.tile([128, MC], mybir.dt.int32, tag="ki")
                    nc.vector.tensor_scalar(
                        out=ki[:], in0=y[:], scalar1=inv2pi, scalar2=None, op0=mult
                    )
                    kf = work.tile([128, MC], F32, tag="kf")
                    nc.vector.tensor_scalar(
                        out=kf[:], in0=ki[:], scalar1=twopi, scalar2=None, op0=mult
                    )
                    w = work.tile([128, MC], F32, tag="wred")
                    nc.vector.tensor_tensor(
                        out=w[:], in0=y[:], in1=kf[:], op=mybir.AluOpType.subtract
                    )
                    sin_t = work.tile([128, MC], F32, tag="trigtmp")
                    nc.scalar.activation(sin_t[:], w[:], sin_f)
                    nc.vector.tensor_scalar_mul(S32[k][:], sin_t[:], w2c[k])
                    if not HESS_FP32:
                        nc.vector.tensor_copy(SH[k][:], S32[k][:])
                    # cos(z) = sin(w + pi/2), wrapped down a period if w > pi/2
                    hi = work.tile([128, MC], F32, tag="hi")
                    nc.vector.tensor_scalar(
                        out=hi[:], in0=w[:], scalar1=float(np.pi / 2), scalar2=-twopi,
                        op0=mybir.AluOpType.is_gt, op1=mult,
                    )
                    wc = work.tile([128, MC], F32, tag="wc")
                    nc.vector.tensor_tensor(
                        out=wc[:], in0=w[:], in1=hi[:], op=mybir.AluOpType.add
                    )
                    cos_t = work.tile([128, MC], F32, tag="trigtmp")
                    nc.scalar.activation(cos_t[:], wc[:], sin_f, bias=pihalf[:])
                    nc.vector.tensor_scalar_mul(C[k][:], cos_t[:], w2c[k])

                # ---- u = ones^T @ S32 + b2 ----
                up = psE.tile([1, MC], F32, tag="u", bufs=1)
                for k in range(2):
                    nc.tensor.matmul(
                        up[:], ones[:], S32[k][:], start=(k == 0), stop=(k == 1)
                    )
                u_sb = work.tile([1, MC], F32, tag="usb")
                nc.vector.tensor_scalar_add(u_sb[:], up[:], b2t)
                nc.sync.dma_start(u_d[:], u_sb[:])

                # ---- g = C^T @ W1 -> DuDt | DuDx ----
                for m in range(NM):
                    ms = slice(m * 128, (m + 1) * 128)
                    gp = psE.tile([128, DP1], F32, tag="g", bufs=2, name=f"gp_{m}")
                    for k in range(2):
                        nc.tensor.matmul(
                            gp[:], C[k][:, ms], w1[k][:], start=(k == 0), stop=(k == 1)
                        )
                    g_sb = work.tile([128, DP1], F32, tag="gsb")
                    nc.vector.tensor_copy(g_sb[:], gp[:])
                    nc.sync.dma_start(dudt_d[ms, :], g_sb[:, 0:1])
                    nc.sync.dma_start(dudx_d[ms, :], g_sb[:, 1:DP1])

            # ---- Hessian: D2[m, jk] = sum_h S[h,m] * P[h, jk] ----
            jpg = GCOLS // D  # j-blocks per group (20)
            with tc.tile_pool(name="psH", bufs=8, space="PSUM") as psH:
                for gi in range(NGROUPS):
                    Pg = []
                    for k in range(2):
                        pt = pP.tile([128, GCOLS], hdt, tag=f"P_{k}", name=f"P_{k}_{gi}")
                        js = slice(gi * jpg, (gi + 1) * jpg)
                        nc.vector.tensor_tensor(
                            out=pt[:].rearrange("p (j l) -> p j l", l=D),
                            in0=negv[k][:, js, None].to_broadcast([128, jpg, D]),
                            in1=vv[k][:, None, :].to_broadcast([128, jpg, D]),
                            op=mult,
                        )
                        Pg.append(pt)
                    for m in range(NM):
                        ms = slice(m * 128, (m + 1) * 128)
                        st = stage_p.tile([128, GCOLS], F32, tag="stage")
                        pss = [
                            psH.tile([128, NCHUNK], F32, tag="hess", name=f"ps_{gi}_{m}_{c}")
                            for c in range(CPG)
                        ]
                        for k in range(2):
                            for c in range(CPG):
                                cs = slice(c * NCHUNK, (c + 1) * NCHUNK)
                                nc.tensor.matmul(
                                    pss[c][:], SH[k][:, ms], Pg[k][:, cs],
                                    start=(k == 0), stop=(k == 1),
                                )
                        for c in range(CPG):
                            cs = slice(c * NCHUNK, (c + 1) * NCHUNK)
                            if c == 0:
                                nc.vector.tensor_copy(st[:, cs], pss[c][:])
                            else:
                                nc.scalar.copy(st[:, cs], pss[c][:])
                        nc.sync.dma_start(
                            d2_d[ms, gi * GCOLS : (gi + 1) * GCOLS], st[:]
                        )

    nc.compile()
    return nc


def kernel(t, X, W1, b1, W2, b2):
    global LAST_EXEC_NS
    t = np.ascontiguousarray(np.asarray(t, dtype=np.float32))
    X = np.ascontiguousarray(np.asarray(X, dtype=np.float32))
    W1 = np.ascontiguousarray(np.asarray(W1, dtype=np.float32))
    b1 = np.asarray(b1, dtype=np.float32).reshape(H)
    W2 = np.asarray(W2, dtype=np.float32).reshape(H)
    b2 = np.asarray(b2, dtype=np.float32).reshape(1)

    xaug_t = np.concatenate([t, X], axis=1).T  # (101, 4096)
    w1t = W1.T  # (101, 256)

    cb = np.zeros((H, DP1 + 3), dtype=np.float32)
    cb[:, 0:DP1] = W1
    cb[:, DP1] = b1
    cb[:, DP1 + 1] = W2
    cb[0, DP1 + 2] = b2[0]

    if "nc" not in _CACHE:
        _CACHE["nc"] = _build()
    nc = _CACHE["nc"]

    in_maps = []
    for i in range(N_CORES):
        ca = np.empty((DP1, MC + H), dtype=np.float32)
        ca[:, 0:MC] = xaug_t[:, i * MC : (i + 1) * MC]
        ca[:, MC:] = w1t
        in_maps.append({"CA": ca, "CB": cb})

    res = run_bass_kernel_spmd(nc, in_maps, list(range(N_CORES)), trace=TRACE)
    LAST_EXEC_NS = res.exec_time_ns

    u = np.concatenate(
        [res.results[i]["u"].reshape(MC, 1) for i in range(N_CORES)], axis=0
    )
    dudt = np.concatenate([res.results[i]["DuDt"] for i in range(N_CORES)], axis=0)
    dudx = np.concatenate([res.results[i]["DuDx"] for i in range(N_CORES)], axis=0)
    d2 = np.concatenate(
        [res.results[i]["D2"].reshape(MC, D, D) for i in range(N_CORES)], axis=0
    )
    return u, dudx, dudt, d2


# revision 11
# speedup vs baseline: 1.6942x; 1.6942x over previous
"""FBSNN net_u_Du kernel for 8 trn2 NeuronCores.

Computes, for u(s) = W2 @ sin(W1 @ s + b1) + b2 with s = [t, x]:
  u            (M,1)
  DuDx = g[:,1:], DuDt = g[:,:1]  with  g = (W2 o cos Z) @ W1
  D2uDx2[m]    = V^T diag(-W2 o sin z_m) V,  V = W1[:,1:]

Key reduction: the per-sample Hessians batch into one dense matmul
  D2[m, j*100+k] = sum_h S[h,m] * P[h, j*100+k]
with S[h,m] = W2[h]*sin(Z[m,h]) and P[h,jk] = -V[h,j]*V[h,k].
Data parallel over M=4096 paths -> 512 per core; weights replicated.

HW notes this shape leans on:
 - HW Sin is only accurate on [-pi, pi]; arguments are range-reduced with
   w = y - 2pi*round(y/2pi) (the DVE f32->i32 cast rounds to nearest).
 - fp32 matmul runs as two PE passes; the Hessian matmul dtype is
   switchable below (fp16 operands stream at full rate, fp32 is exact).
 - all inputs are packed into one (128, 976) tensor loaded by a single
   SWDGE DMA; HWDGE 2D loads serialize on one SDMA engine (~15us).
"""

import numpy as np

import concourse.bacc as bacc
import concourse.mybir as mybir
import concourse.tile as tile
from concourse.bass_utils import run_bass_kernel_spmd

N_CORES = 8
M_FULL = 4096
MC = M_FULL // N_CORES  # 512 paths per core
D = 100
DP1 = D + 1  # 101
H = 256  # hidden width
F32 = mybir.dt.float32
F16 = mybir.dt.float16

HESS_FP32 = False  # False: fp16 Hessian matmul (2x faster PE, ~5e-4 D2 err)

NCHUNK = 500  # matmul free-dim per Hessian matmul (<=512 fp32 PSUM bank)
CPG = 4  # chunks per DMA group
GCOLS = NCHUNK * CPG  # 2000
NGROUPS = (D * D) // GCOLS  # 5
NM = MC // 128  # 4 m-chunks of 128 paths

# set by test harness to profile; kernel() records exec time here
TRACE = False
LAST_EXEC_NS = None

_CACHE = {}


def _build():
    nc = bacc.Bacc(None, target_bir_lowering=False, debug=False)
    sin_f = mybir.ActivationFunctionType.Sin
    mult = mybir.AluOpType.mult
    hdt = F32 if HESS_FP32 else F16

    # One packed input, (128, 976):
    #   cols 0:768    [XT | W1T] content on partitions 0:101 (rest zero)
    #   cols 768:872  CB chunk0 = [W1[0:128] | b1 | W2 | b2pad]
    #   cols 872:976  CB chunk1 = [W1[128:256] | b1 | W2 | 0]
    # Loaded with a single SWDGE DMA (sprays across all 16 SDMA engines;
    # HWDGE serializes multi-row 2D loads onto one engine).
    CBW = DP1 + 3
    in_d = nc.dram_tensor("IN", [128, MC + H + 2 * CBW], F32, kind="ExternalInput")

    u_d = nc.dram_tensor("u", [1, MC], F32, kind="ExternalOutput")
    dudt_d = nc.dram_tensor("DuDt", [MC, 1], F32, kind="ExternalOutput")
    dudx_d = nc.dram_tensor("DuDx", [MC, D], F32, kind="ExternalOutput")
    d2_d = nc.dram_tensor("D2", [MC, D * D], F32, kind="ExternalOutput")

    with tile.TileContext(nc) as tc:
        with (
            tc.tile_pool(name="const", bufs=1) as const,
            tc.tile_pool(name="work", bufs=2) as work,
            tc.tile_pool(name="pP", bufs=3) as pP,
            tc.tile_pool(name="stage", bufs=4) as stage_p,
        ):
            # ---- load inputs: one sprayed DMA ----
            inp = const.tile([128, MC + H + 2 * CBW], F32)
            nc.gpsimd.dma_start(inp[:], in_d[:])
            ca = inp[0:DP1, :]
            cb = [inp[:, MC + H + k * CBW : MC + H + (k + 1) * CBW] for k in range(2)]
            xt = ca[:, 0:MC]
            w1t = ca[:, MC : MC + H]
            w1 = [cb[k][:, 0:DP1] for k in range(2)]
            b1c = [cb[k][:, DP1 : DP1 + 1] for k in range(2)]
            w2c = [cb[k][:, DP1 + 1 : DP1 + 2] for k in range(2)]
            b2t = cb[0][0:1, DP1 + 2 : DP1 + 3]

            ones = const.tile([128, 1], F32)
            nc.vector.memset(ones[:], 1.0)
            pihalf = const.tile([128, 1], F32)
            nc.vector.memset(pihalf[:], float(np.pi / 2))

            # hess-dtype views of V = W1[:, 1:] and -V
            vv = [const.tile([128, D], hdt, tag=f"vv_{k}", name=f"vv_{k}") for k in range(2)]
            negv = [const.tile([128, D], hdt, tag=f"negv_{k}", name=f"negv_{k}") for k in range(2)]
            for k in range(2):
                nc.vector.tensor_copy(vv[k][:], w1[k][:, 1:DP1])
                nc.vector.tensor_scalar_mul(negv[k][:], w1[k][:, 1:DP1], -1.0)

            S32 = [const.tile([128, MC], F32, tag=f"S32_{k}", name=f"S32_{k}") for k in range(2)]
            SH = S32 if HESS_FP32 else [
                const.tile([128, MC], F16, tag=f"S16_{k}", name=f"S16_{k}") for k in range(2)
            ]
            C = [const.tile([128, MC], F32, tag=f"C_{k}", name=f"C_{k}") for k in range(2)]
            inv2pi = float(1.0 / (2.0 * np.pi))
            twopi = float(2.0 * np.pi)

            with tc.tile_pool(name="psE", bufs=2, space="PSUM") as psE:
                # ---- trig: Z^T = W1 @ [t,X]^T; S = W2*sin(Z), C = W2*cos(Z) ----
                for k in range(2):
                    ztp = psE.tile([128, MC], F32, tag="zt")
                    nc.tensor.matmul(
                        ztp[:], w1t[:, k * 128 : (k + 1) * 128], xt[:],
                        start=True, stop=True,
                    )
                    y = work.tile([128, MC], F32, tag="y")
                    nc.vector.tensor_scalar_add(y[:], ztp[:], b1c[k])
                    ki = work.tile([128, MC], mybir.dt.int32, tag="ki")
                    nc.vector.tensor_scalar(
                        out=ki[:], in0=y[:], scalar1=inv2pi, scalar2=None, op0=mult
                    )
                    kf = work.tile([128, MC], F32, tag="kf")
                    nc.vector.tensor_scalar(
                        out=kf[:], in0=ki[:], scalar1=twopi, scalar2=None, op0=mult
                    )
                    w = work.tile([128, MC], F32, tag="wred")
                    nc.vector.tensor_tensor(
                        out=w[:], in0=y[:], in1=kf[:], op=mybir.AluOpType.subtract
                    )
                    sin_t = work.tile([128, MC], F32, tag="trigtmp")
                    nc.scalar.activation(sin_t[:], w[:], sin_f)
                    nc.vector.tensor_scalar_mul(S32[k][:], sin_t[:], w2c[k])
                    if not HESS_FP32:
                        nc.vector.tensor_copy(SH[k][:], S32[k][:])
                    # cos(z) = sin(w + pi/2), wrapped down a period if w > pi/2
                    hi = work.tile([128, MC], F32, tag="hi")
                    nc.vector.tensor_scalar(
                        out=hi[:], in0=w[:], scalar1=float(np.pi / 2), scalar2=-twopi,
                        op0=mybir.AluOpType.is_gt, op1=mult,
                    )
                    wc = work.tile([128, MC], F32, tag="wc")
                    nc.vector.tensor_tensor(
                        out=wc[:], in0=w[:], in1=hi[:], op=mybir.AluOpType.add
                    )
                    cos_t = work.tile([128, MC], F32, tag="trigtmp")
                    nc.scalar.activation(cos_t[:], wc[:], sin_f, bias=pihalf[:])
                    nc.vector.tensor_scalar_mul(C[k][:], cos_t[:], w2c[k])

                # ---- u = ones^T @ S32 + b2 ----
                up = psE.tile([1, MC], F32, tag="u", bufs=1)
                for k in range(2):
                    nc.tensor.matmul(
                        up[:], ones[:], S32[k][:], start=(k == 0), stop=(k == 1)
                    )
                u_sb = work.tile([1, MC], F32, tag="usb")
                nc.vector.tensor_scalar_add(u_sb[:], up[:], b2t)
                nc.sync.dma_start(u_d[:], u_sb[:])

                # ---- g = C^T @ W1 -> DuDt | DuDx ----
                for m in range(NM):
                    ms = slice(m * 128, (m + 1) * 128)
                    gp = psE.tile([128, DP1], F32, tag="g", bufs=2, name=f"gp_{m}")
                    for k in range(2):
                        nc.tensor.matmul(
                            gp[:], C[k][:, ms], w1[k][:], start=(k == 0), stop=(k == 1)
                        )
                    g_sb = work.tile([128, DP1], F32, tag="gsb")
                    nc.vector.tensor_copy(g_sb[:], gp[:])
                    nc.sync.dma_start(dudt_d[ms, :], g_sb[:, 0:1])
                    nc.sync.dma_start(dudx_d[ms, :], g_sb[:, 1:DP1])

            # ---- Hessian: D2[m, jk] = sum_h S[h,m] * P[h, jk] ----
            jpg = GCOLS // D  # j-blocks per group (20)
            with tc.tile_pool(name="psH", bufs=8, space="PSUM") as psH:
                for gi in range(NGROUPS):
                    Pg = []
                    for k in range(2):
                        pt = pP.tile([128, GCOLS], hdt, tag=f"P_{k}", name=f"P_{k}_{gi}")
                        js = slice(gi * jpg, (gi + 1) * jpg)
                        nc.vector.tensor_tensor(
                            out=pt[:].rearrange("p (j l) -> p j l", l=D),
                            in0=negv[k][:, js, None].to_broadcast([128, jpg, D]),
                            in1=vv[k][:, None, :].to_broadcast([128, jpg, D]),
                            op=mult,
                        )
                        Pg.append(pt)
                    for m in range(NM):
                        ms = slice(m * 128, (m + 1) * 128)
                        st = stage_p.tile([128, GCOLS], F32, tag="stage")
                        pss = [
                            psH.tile([128, NCHUNK], F32, tag="hess", name=f"ps_{gi}_{m}_{c}")
                            for c in range(CPG)
                        ]
                        for k in range(2):
                            for c in range(CPG):
                                cs = slice(c * NCHUNK, (c + 1) * NCHUNK)
                                nc.tensor.matmul(
                                    pss[c][:], SH[k][:, ms], Pg[k][:, cs],
                                    start=(k == 0), stop=(k == 1),
                                )
                        for c in range(CPG):
                            cs = slice(c * NCHUNK, (c + 1) * NCHUNK)
                            if c == 0:
                                nc.vector.tensor_copy(st[:, cs], pss[c][:])
                            else:
                                nc.scalar.copy(st[:, cs], pss[c][:])
                        nc.sync.dma_start(
                            d2_d[ms, gi * GCOLS : (gi + 1) * GCOLS], st[:]
                        )

    nc.compile()
    return nc


def kernel(t, X, W1, b1, W2, b2):
    global LAST_EXEC_NS
    t = np.ascontiguousarray(np.asarray(t, dtype=np.float32))
    X = np.ascontiguousarray(np.asarray(X, dtype=np.float32))
    W1 = np.ascontiguousarray(np.asarray(W1, dtype=np.float32))
    b1 = np.asarray(b1, dtype=np.float32).reshape(H)
    W2 = np.asarray(W2, dtype=np.float32).reshape(H)
    b2 = np.asarray(b2, dtype=np.float32).reshape(1)

    xaug_t = np.concatenate([t, X], axis=1).T  # (101, 4096)
    w1t = W1.T  # (101, 256)

    CBW = DP1 + 3
    base = np.zeros((128, MC + H + 2 * CBW), dtype=np.float32)
    base[0:DP1, MC : MC + H] = w1t
    for k in range(2):
        c0 = MC + H + k * CBW
        base[:, c0 : c0 + DP1] = W1[k * 128 : (k + 1) * 128, :]
        base[:, c0 + DP1] = b1[k * 128 : (k + 1) * 128]
        base[:, c0 + DP1 + 1] = W2[k * 128 : (k + 1) * 128]
    base[0, MC + H + DP1 + 2] = b2[0]

    if "nc" not in _CACHE:
        _CACHE["nc"] = _build()
    nc = _CACHE["nc"]

    in_maps = []
    for i in range(N_CORES):
        pk = base.copy()
        pk[0:DP1, 0:MC] = xaug_t[:, i * MC : (i + 1) * MC]
        in_maps.append({"IN": pk})

    res = run_bass_kernel_spmd(nc, in_maps, list(range(N_CORES)), trace=TRACE)
    LAST_EXEC_NS = res.exec_time_ns

    u = np.concatenate(
        [res.results[i]["u"].reshape(MC, 1) for i in range(N_CORES)], axis=0
    )
    dudt = np.concatenate([res.results[i]["DuDt"] for i in range(N_CORES)], axis=0)
    dudx = np.concatenate([res.results[i]["DuDx"] for i in range(N_CORES)], axis=0)
    d2 = np.concatenate(
        [res.results[i]["D2"].reshape(MC, D, D) for i in range(N_CORES)], axis=0
    )
    return u, dudx, dudt, d2
